# revision 34
# baseline (speedup 1.0000x reference)
"""GRU cell (AnotherGRUCell) on 8 TRN2 NeuronCores.

Strategy: pure data-parallel over batch (8192 rows -> 1024 rows/core),
weights replicated. No collectives.

All on-chip compute is in TRANSPOSED layout (units on the partition axis,
batch on the free axis), as in the bf16 baseline. New here: most of the
GEMM work runs in fp8-e4m3 with perf_mode=DoubleRow (2 k-tiles per PE
instruction), which roughly doubles PE matmul throughput. fp8
quantization noise is steered by a per-unit precision config chosen via
a host-side numpy simulation against the 2e-2 rel-err gate:

  - r gates (cols 0..15):     x@Wr + h@Wr fully fp8 (error is attenuated
                              through r*h -> cand -> (1-u) weighting)
  - u gates (cols 16..31):    first N8_XU/N8_HU k-tiles of x/h in fp8,
                              rest bf16 (u multiplies h directly in the
                              output, so u noise is expensive)
  - cand (r*h)@Wh3:           fully fp8 (attenuated like r)
  - cand x@Wi3:               bf16 (tanh pre-act noise is expensive)

All weights (both dtypes) are pre-scaled by S_W=32 on the host so fp8
sees a ~unit-std distribution, and every PSUM accumulation has one
uniform scale that is divided out for free inside the ScalarE
activation (out = sigmoid/tanh(psum * 1/S_W + bias)).

fp8 x/h/rh operands live in PAIR tiles [128, 2, 1024] so each DoubleRow
matmul gets its required 3D AP [128, 2, free] (pair-dim step % 16 == 0)
while startup DMAs keep per-pair dependency granularity.
"""

import numpy as np
import ml_dtypes

import concourse.bacc as bacc
import concourse.tile as tile
import concourse.mybir as mybir
from concourse.bass_utils import run_bass_kernel_spmd

N_CORES = 8
UNITS = 2048
IN_DIM = 2048
BATCH = 8192
B_LOC = BATCH // N_CORES  # 1024 batch rows per core

P = 128
KT_X = IN_DIM // P           # 16 k-tiles of x
KT_H = UNITS // P            # 16 k-tiles of h
KT = KT_X + KT_H             # 32 contraction k-tiles for [x; h]
NT_G = (2 * UNITS) // P      # 32 gate col-tiles (r: 0..15, u: 16..31)
NT_C = UNITS // P            # 16 candidate col-tiles
M_CHUNK = 512
MC = B_LOC // M_CHUNK        # 2 moving chunks per core

# Precision config: number of leading k-tiles (of 16) computed in fp8
# DoubleRow for the u-gate x/h operands. r gates and (r*h)@Wh3 are fully
# fp8; the candidate x@Wi3 is fully bf16. Must be even (DoubleRow pairs).
# Host-sim predicted rel err (matches HW to ~5 digits): 4 -> 1.571e-2,
# 8 -> 1.811e-2, 10 -> 1.920e-2 against the 2e-2 gate.
N8_XU = 10
N8_HU = 10
S_W = 32.0
S_INV = float(1.0 / S_W)

BF16 = mybir.dt.bfloat16
F32 = mybir.dt.float32
FP8 = mybir.dt.float8e4
NP_BF16 = ml_dtypes.bfloat16
NP_FP8 = ml_dtypes.float8_e4m3  # IEEE-style e4m3, max 240 == TRN FP8_EXP4
DR = mybir.MatmulPerfMode.DoubleRow

_CACHED_NC = None

# test.py sets TRACE=True to capture the NTFF profile (exec_time_ns +
# perfetto trace); the graded path leaves it off. LAST_RESULTS holds the
# BassKernelResults of the most recent run.
TRACE = False
LAST_RESULTS = None


def _build():
    nc = bacc.Bacc("TRN2", target_bir_lowering=False, debug=False)

    # fp8 transposed inputs, one [128, 1024] slice per k-tile
    x8d = nc.dram_tensor("x8", [KT_X, P, B_LOC], FP8, kind="ExternalInput")
    h8d = nc.dram_tensor("h8", [KT_H, P, B_LOC], FP8, kind="ExternalInput")
    # bf16 transposed inputs (u-gate bf16 part, cand x part, epilogues)
    xbd = nc.dram_tensor("xb", [KT_X, P, B_LOC], BF16, kind="ExternalInput")
    hbd = nc.dram_tensor("hb", [KT_H, P, B_LOC], BF16, kind="ExternalInput")
    # weights, pre-scaled by S_W, packed per col-tile as [128p, nkt, 128c]
    n8u = N8_XU + N8_HU
    nbu = KT - n8u
    w_r8 = nc.dram_tensor("w_r8", [NT_C, P, KT, P], FP8, kind="ExternalInput")
    # fused-phase slabs merged by dtype (one DMA + one PE first-use wait
    # per dtype per col-tile): fp8 = [u-gate fp8 k-tiles | rh k-tiles],
    # bf16 = [u-gate bf16 k-tiles | cand-x k-tiles]
    w_uc8 = nc.dram_tensor(
        "w_uc8", [NT_C, P, n8u + KT_H, P], FP8, kind="ExternalInput"
    )
    w_ucb = nc.dram_tensor(
        "w_ucb", [NT_C, P, nbu + KT_X, P], BF16, kind="ExternalInput"
    )
    # biases transposed: one [128, n_tiles] tensor per gate set -> 1 DMA each
    b_g = nc.dram_tensor("b_g", [P, NT_G], F32, kind="ExternalInput")
    b_c = nc.dram_tensor("b_c", [P, NT_C], F32, kind="ExternalInput")
    out = nc.dram_tensor("out", [NT_C, P, B_LOC], F32, kind="ExternalOutput")

    SIG = mybir.ActivationFunctionType.Sigmoid
    TANH = mybir.ActivationFunctionType.Tanh

    NPAIR_X = KT_X // 2
    NPAIR_H = KT_H // 2

    with tile.TileContext(nc) as tc:
        with (
            tc.tile_pool(name="resident", bufs=1) as res,
            tc.tile_pool(name="wslab", bufs=2) as wp,
            tc.tile_pool(name="psum", bufs=8, space="PSUM") as pp,
            tc.tile_pool(name="stage", bufs=2) as sp,
            tc.tile_pool(name="bias", bufs=1) as bp,
        ):
            # fp8 pair tiles: [128, 2, 1024]; pair q holds k-tiles 2q, 2q+1
            x8_pairs = [
                res.tile([P, 2, B_LOC], FP8, tag=f"x8{q}", name=f"x8{q}")
                for q in range(NPAIR_X)
            ]
            h8_pairs = [
                res.tile([P, 2, B_LOC], FP8, tag=f"h8{q}", name=f"h8{q}")
                for q in range(NPAIR_H)
            ]
            rh8_pairs = [
                res.tile([P, 2, B_LOC], FP8, tag=f"rh{q}", name=f"rh{q}")
                for q in range(NPAIR_H)
            ]
            # bf16 per-k-tile tiles
            xb_tiles = [
                res.tile([P, B_LOC], BF16, tag=f"xb{j}", name=f"xb{j}")
                for j in range(KT_X)
            ]
            hb_tiles = [
                res.tile([P, B_LOC], BF16, tag=f"hb{j}", name=f"hb{j}")
                for j in range(KT_H)
            ]
            # u gates are transient: phases U and C are interleaved per
            # col-tile, so u lives only from its sigmoid to the combine a
            # few us later (saves 30KB/partition of SBUF, spent on deeper
            # weight-slab prefetch and a wider startup interleave).

            # PE warm-up: the HAM clock gate holds the PE at 1.2 GHz until
            # it has been busy ~3.4us; fill the pre-first-matmul window
            # with dummy matmuls so the PE is un-throttled when real data
            # lands (same trick as the bf16 baseline).
            warm_src = sp.tile(
                [P, M_CHUNK], BF16, tag="warm", name="warm_src", bufs=1
            )
            nc.gpsimd.memset(warm_src[:], 0.0)
            warm_ps = pp.tile([P, M_CHUNK], F32, tag="psum", name="warm_ps")
            for w in range(8):
                nc.tensor.matmul(
                    warm_ps[:],
                    warm_src[:, :P],
                    warm_src[:],
                    start=(w == 0),
                    stop=(w == 7),
                )

            # Startup DMAs in exact consumption order of the first r-gate
            # col-tile pair, interleaved across both HWDGE rings.
            # Graduated chunk sizes (in k-tiles over the 32-long [x; h]
            # sequence); all chunk boundaries are even so DoubleRow pairs
            # never straddle a chunk.
            CHUNKS = [2, 6, 8, 8, 8]
            CB = [0, 2, 8, 16, 24, 32]  # chunk k-tile boundaries
            NT0 = 6  # r col-tiles in the two-pass startup interleave
            ws_first = [[None] * len(CHUNKS) for _ in range(NT0)]  # [t][chunk]
            src_dma = {}  # k-slot -> (engine, dst ap, src ap)
            for j in range(KT_X):
                eng = nc.sync if j % 2 == 0 else nc.scalar
                src_dma[j] = (eng, x8_pairs[j // 2][:, j % 2, :], x8d[j, :, :])
            for j in range(KT_H):
                eng = nc.scalar if j % 2 == 0 else nc.sync
                src_dma[KT_X + j] = (
                    eng, h8_pairs[j // 2][:, j % 2, :], h8d[j, :, :]
                )
            pre_ws = {}
            for c, cw in enumerate(CHUNKS):
                if c == 0:
                    # The very first matmul's operands go FIRST in each
                    # ring queue: x8 pair 0 then the first weight chunk.
                    for j in range(CB[0], CB[1]):
                        eng, dst, src = src_dma[j]
                        eng.dma_start(dst, src)
                if c == len(CHUNKS) - 1:
                    # Sneak the first steady-state r slabs in ahead of
                    # the last startup chunk: t=NT0's slab gates the PE
                    # right after the interleaved block and must not sit
                    # behind the bf16 input stream.
                    for t in (NT0, NT0 + 1):
                        ws = wp.tile([P, KT, P], FP8, tag="wr", name=f"wr{t}", bufs=3)
                        (nc.sync if t % 2 == 0 else nc.scalar).dma_start(
                            ws[:], w_r8[t, :, :, :]
                        )
                        pre_ws[t] = ws
                for t in range(NT0):
                    w = wp.tile(
                        [P, cw, P], FP8, tag=f"wr{t}_{c}", name=f"wr{t}_{c}",
                        bufs=1,
                    )
                    (nc.sync if t % 2 == 0 else nc.scalar).dma_start(
                        w[:], w_r8[t, :, CB[c]:CB[c + 1], :]
                    )
                    ws_first[t][c] = w
                if c > 0:
                    for j in range(CB[c], CB[c + 1]):
                        eng, dst, src = src_dma[j]
                        eng.dma_start(dst, src)

            # Biases + the early bf16 h tiles (needed by the first r
            # epilogues ~30us in) go on the SWDGE queue: the two HWDGE
            # rings deliver ~100GB/s each and are fully booked with the
            # startup x8/h8/weight traffic that gates the PE.
            bg_all = bp.tile([P, NT_G], F32, tag="bg", name="bg_all")
            nc.gpsimd.dma_start(bg_all[:], b_g[:, :])
            bc_all = bp.tile([P, NT_C], F32, tag="bc", name="bc_all")
            nc.gpsimd.dma_start(bc_all[:], b_c[:, :])
            for j in range(NT0 + 2):
                nc.gpsimd.dma_start(hb_tiles[j][:], hbd[j, :, :])

            all_pairs = x8_pairs + h8_pairs  # 16 fp8 pair tiles = 32 k-tiles

            def act_r(t, m, ps):
                """r epilogue: rh8[t] = sigmoid(ps/S_W + b) * h  (fp8)."""
                ms = slice(m * M_CHUNK, (m + 1) * M_CHUNK)
                rt = sp.tile([P, M_CHUNK], BF16, tag="rtmp", name=f"r{t}_{m}")
                nc.scalar.activation(
                    rt[:], ps[:], SIG, bias=bg_all[:, t:t + 1], scale=S_INV
                )
                nc.vector.tensor_mul(
                    rh8_pairs[t // 2][:, t % 2, ms], rt[:], hb_tiles[t][:, ms]
                )

            # ---- Phase R: r gates (cols 0..15), fully fp8 DoubleRow ------
            # The first NT0 col-tiles are block-interleaved over the
            # startup chunks (NT0*2 psum groups): the startup is input-
            # bandwidth-bound (~6MB before steady state), so the PE needs
            # ~NT0 tiles of matmul work per arriving chunk to stay busy.
            # Two-pass startup: pass 1 computes only the m=0 chunk of the
            # NT0 interleaved tiles, paced by the arriving chunks (the
            # startup is input-bandwidth-bound); pass 2 (m=1) then reuses
            # the chunk weights already in SBUF -- pure PE work with zero
            # new DMA, hiding the hb/xb/steady-slab stream behind it.
            pss0 = [
                pp.tile([P, M_CHUNK], F32, tag="psum", name=f"psg0_{i}")
                for i in range(NT0)
            ]
            for c in range(len(CHUNKS)):
                q0, q1 = CB[c] // 2, CB[c + 1] // 2
                for t in range(NT0):
                    for qq in range(q0, q1):
                        jj = qq - q0  # pair index within this chunk's slab
                        nc.tensor.matmul(
                            pss0[t][:],
                            ws_first[t][c][:, 2 * jj:2 * jj + 2, :],
                            all_pairs[qq][:, 0:2, 0:M_CHUNK],
                            start=(qq == 0),
                            stop=(qq == KT // 2 - 1),
                            perf_mode=DR,
                        )
            for t in range(NT0):
                act_r(t, 0, pss0[t])
            for t in range(NT0):
                ps1 = pp.tile([P, M_CHUNK], F32, tag="psum", name=f"psg1_{t}")
                ms = slice(M_CHUNK, 2 * M_CHUNK)
                for qq in range(KT // 2):
                    c = next(i for i in range(len(CHUNKS))
                             if CB[i] <= 2 * qq < CB[i + 1])
                    jj = qq - CB[c] // 2
                    nc.tensor.matmul(
                        ps1[:],
                        ws_first[t][c][:, 2 * jj:2 * jj + 2, :],
                        all_pairs[qq][:, 0:2, ms],
                        start=(qq == 0),
                        stop=(qq == KT // 2 - 1),
                        perf_mode=DR,
                    )
                act_r(t, 1, ps1)

            # Steady-state r cols: one fp8 slab [128, 32, 128] per col-tile,
            # m-interleaved so consecutive matmuls share the stationary
            # weight pair (one 256-col LDWEIGHTS per 2 matmuls).
            for t in range(NT0, NT_C):
                if t in pre_ws:
                    ws = pre_ws[t]
                else:
                    ws = wp.tile([P, KT, P], FP8, tag="wr", name=f"wr{t}", bufs=3)
                    (nc.sync if t % 2 == 0 else nc.scalar).dma_start(
                        ws[:], w_r8[t, :, :, :]
                    )
                # pace the bf16 inputs behind the slab they follow:
                # hb[t] lands ~1 col-tile before its epilogue needs it,
                # xb streams in over the back half of the r phase (it is
                # first read in the fused u/cand phase).
                if t < KT_H - 2:
                    (nc.scalar if t % 2 == 0 else nc.sync).dma_start(
                        hb_tiles[t + 2][:], hbd[t + 2, :, :]
                    )
                if t >= 8:
                    j0 = 2 * (t - 8)
                    (nc.scalar if t % 2 == 0 else nc.sync).dma_start(
                        xb_tiles[j0][:], xbd[j0, :, :]
                    )
                    (nc.sync if t % 2 == 0 else nc.scalar).dma_start(
                        xb_tiles[j0 + 1][:], xbd[j0 + 1, :, :]
                    )
                psl = [
                    pp.tile([P, M_CHUNK], F32, tag="psum", name=f"psr{t}_{m}")
                    for m in range(MC)
                ]
                for q in range(KT // 2):
                    for m in range(MC):
                        ms = slice(m * M_CHUNK, (m + 1) * M_CHUNK)
                        nc.tensor.matmul(
                            psl[m][:],
                            ws[:, 2 * q:2 * q + 2, :],
                            all_pairs[q][:, 0:2, ms],
                            start=(q == 0),
                            stop=(q == KT // 2 - 1),
                            perf_mode=DR,
                        )
                for m in range(MC):
                    act_r(t, m, psl[m])

            # ---- Fused phase U+C: per col-tile t, compute the u gate
            # (cols 16+t, mixed fp8/bf16) and immediately the candidate +
            # output combine for the same t. u_t lives only a few us in a
            # rotating stage tile instead of 32KB of resident SBUF.
            # psum_c = (r*h)@Wh3 (fp8 DR) + x@Wi3 (bf16);
            # h_t = u * (h - cand) + cand
            def uc_slabs(t):
                w8t = wp.tile(
                    [P, n8u + KT_H, P], FP8, tag="wuc8", name=f"wuc8_{t}",
                    bufs=3,
                )
                (nc.sync if t % 2 == 0 else nc.scalar).dma_start(
                    w8t[:], w_uc8[t, :, :, :]
                )
                wbt = wp.tile(
                    [P, nbu + KT_X, P], BF16, tag="wucb", name=f"wucb_{t}",
                    bufs=3,
                )
                (nc.scalar if t % 2 == 0 else nc.sync).dma_start(
                    wbt[:], w_ucb[t, :, :, :]
                )
                return w8t, wbt

            def u_accum(w8, wb, psl):
                n_mm = n8u // 2 + nbu  # accumulation steps per m-chunk
                step = 0
                for q in range(N8_XU // 2):
                    for m in range(MC):
                        ms = slice(m * M_CHUNK, (m + 1) * M_CHUNK)
                        nc.tensor.matmul(
                            psl[m][:],
                            w8[:, 2 * q:2 * q + 2, :],
                            x8_pairs[q][:, 0:2, ms],
                            start=(step == 0),
                            stop=(step == n_mm - 1),
                            perf_mode=DR,
                        )
                    step += 1
                for q in range(N8_HU // 2):
                    off = N8_XU + 2 * q
                    for m in range(MC):
                        ms = slice(m * M_CHUNK, (m + 1) * M_CHUNK)
                        nc.tensor.matmul(
                            psl[m][:],
                            w8[:, off:off + 2, :],
                            h8_pairs[q][:, 0:2, ms],
                            start=(step == 0),
                            stop=(step == n_mm - 1),
                            perf_mode=DR,
                        )
                    step += 1
                # bf16 part: x k-tiles N8_XU..15, then h k-tiles N8_HU..15
                for i, src in enumerate(
                    [xb_tiles[j] for j in range(N8_XU, KT_X)]
                    + [hb_tiles[j] for j in range(N8_HU, KT_H)]
                ):
                    for m in range(MC):
                        ms = slice(m * M_CHUNK, (m + 1) * M_CHUNK)
                        nc.tensor.matmul(
                            psl[m][:],
                            wb[:, i, :],
                            src[:, ms],
                            start=(step == 0),
                            stop=(step == n_mm - 1),
                        )
                    step += 1

            def cand_accum(w8, wb, psl):
                # c-phase k-tiles sit after the u-phase ones in the merged
                # slabs: fp8 at offset n8u, bf16 at offset nbu.
                n_mm = KT_H // 2 + KT_X
                step = 0
                for q in range(KT_H // 2):
                    off = n8u + 2 * q
                    for m in range(MC):
                        ms = slice(m * M_CHUNK, (m + 1) * M_CHUNK)
                        nc.tensor.matmul(
                            psl[m][:],
                            w8[:, off:off + 2, :],
                            rh8_pairs[q][:, 0:2, ms],
                            start=(step == 0),
                            stop=(step == n_mm - 1),
                            perf_mode=DR,
                        )
                    step += 1
                for j in range(KT_X):
                    for m in range(MC):
                        ms = slice(m * M_CHUNK, (m + 1) * M_CHUNK)
                        nc.tensor.matmul(
                            psl[m][:],
                            wb[:, nbu + j, :],
                            xb_tiles[j][:, ms],
                            start=(step == 0),
                            stop=(step == n_mm - 1),
                        )
                    step += 1

            def cand_epilogue(t, m, ut, ps):
                ms = slice(m * M_CHUNK, (m + 1) * M_CHUNK)
                cand = sp.tile([P, M_CHUNK], F32, tag="cand", name=f"c{t}_{m}")
                nc.scalar.activation(
                    cand[:], ps[:], TANH, bias=bc_all[:, t:t + 1], scale=S_INV
                )
                d = sp.tile([P, M_CHUNK], F32, tag="d", name=f"d{t}_{m}")
                nc.vector.tensor_sub(d[:], hb_tiles[t][:, ms], cand[:])
                d2 = sp.tile([P, M_CHUNK], F32, tag="d2", name=f"d2{t}_{m}")
                nc.vector.tensor_mul(d2[:], ut[:, ms], d[:])
                ht = sp.tile([P, M_CHUNK], F32, tag="ht", name=f"ht{t}_{m}")
                nc.vector.tensor_add(ht[:], d2[:], cand[:])
                # Outs split across both rings; tile t+1's slab DMAs are
                # issued BEFORE these in program order, so outputs never
                # delay the weight stream (run-2's 13us tail) and don't
                # drain on the slow SWDGE queue (run-3's 17us tail).
                (nc.sync if m == 0 else nc.scalar).dma_start(
                    out[t, :, ms], ht[:]
                )

            def cand_epilogue_narrow(t, m, half, ut, ps):
                """256-wide epilogue slice: on the last tile the ACT->DVE
                ->DMA chain is the post-final-matmul drain, so pipeline it
                at half-chunk granularity to shorten the critical path."""
                HW = M_CHUNK // 2
                ms = slice(m * M_CHUNK + half * HW, m * M_CHUNK + (half + 1) * HW)
                ps_sl = ps[:, half * HW:(half + 1) * HW]
                cand = sp.tile([P, HW], F32, tag="cand", name=f"cn{m}_{half}")
                nc.scalar.activation(
                    cand[:], ps_sl, TANH, bias=bc_all[:, t:t + 1], scale=S_INV
                )
                d = sp.tile([P, HW], F32, tag="d", name=f"dn{m}_{half}")
                nc.vector.tensor_sub(d[:], hb_tiles[t][:, ms], cand[:])
                d2 = sp.tile([P, HW], F32, tag="d2", name=f"d2n{m}_{half}")
                nc.vector.tensor_mul(d2[:], ut[:, ms], d[:])
                ht = sp.tile([P, HW], F32, tag="ht", name=f"htn{m}_{half}")
                nc.vector.tensor_add(ht[:], d2[:], cand[:])
                (nc.sync if half == 0 else nc.scalar).dma_start(
                    out[t, :, ms], ht[:]
                )

            slabs = {0: uc_slabs(0)}
            for t in range(NT_C):
                if t + 1 < NT_C:
                    slabs[t + 1] = uc_slabs(t + 1)
                w8t, wbt = slabs.pop(t)
                ut = sp.tile([P, B_LOC], BF16, tag="ut", name=f"ut{t}")
                psu = [
                    pp.tile([P, M_CHUNK], F32, tag="psum", name=f"psu{t}_{m}")
                    for m in range(MC)
                ]
                u_accum(w8t, wbt, psu)
                for m in range(MC):
                    ms = slice(m * M_CHUNK, (m + 1) * M_CHUNK)
                    nc.scalar.activation(
                        ut[:, ms], psu[m][:], SIG,
                        bias=bg_all[:, NT_C + t:NT_C + t + 1], scale=S_INV,
                    )
                psc = [
                    pp.tile([P, M_CHUNK], F32, tag="psum", name=f"psc{t}_{m}")
                    for m in range(MC)
                ]
                cand_accum(w8t, wbt, psc)
                if t < NT_C - 1:
                    for m in range(MC):
                        cand_epilogue(t, m, ut, psc[m])
                else:
                    for m in range(MC):
                        for half in range(2):
                            cand_epilogue_narrow(t, m, half, ut, psc[m])

    nc.compile()
    return nc


def _get_nc():
    global _CACHED_NC
    if _CACHED_NC is None:
        _CACHED_NC = _build()
    return _CACHED_NC


def _ct_blocks(w):
    """[K, N] -> [N/128 col-tiles, K/128 k-tiles, 128p, 128c] blocks."""
    K, N = w.shape
    return np.ascontiguousarray(
        w.reshape(K // P, P, N // P, P).transpose(2, 0, 1, 3)
    )


def _slab(blocks, ct, sel, np_dtype):
    """Pack k-tiles `sel` of col-tile ct into [128p, len(sel), 128c]."""
    a = blocks[ct][sel]  # [nkt, 128p, 128c]
    return np.ascontiguousarray(a.transpose(1, 0, 2)).astype(np_dtype)


def kernel(x_t, h_tm1, input_weight, hidden_state_weight, bias):
    x_t = np.asarray(x_t, dtype=np.float32)
    h_tm1 = np.asarray(h_tm1, dtype=np.float32)
    input_weight = np.asarray(input_weight, dtype=np.float32)
    hidden_state_weight = np.asarray(hidden_state_weight, dtype=np.float32)
    bias = np.asarray(bias, dtype=np.float32)

    u = UNITS
    # Gate weights: [x; h] @ [Wi[:, :2u]; Wh[:, :2u]], pre-scaled by S_W
    w_gate = np.concatenate(
        [input_weight[:, : 2 * u], hidden_state_weight[:, : 2 * u]], axis=0
    ) * np.float32(S_W)  # [4096, 4096]
    w_cand = np.concatenate(
        [input_weight[:, 2 * u:], hidden_state_weight[:, 2 * u:]], axis=0
    ) * np.float32(S_W)  # [4096, 2048]

    bg = _ct_blocks(w_gate)   # [32 ct, 32 kt, 128, 128]
    bc = _ct_blocks(w_cand)   # [16 ct, 32 kt, 128, 128]

    kt_all = list(range(KT))
    sel_u8 = list(range(N8_XU)) + list(range(KT_X, KT_X + N8_HU))
    sel_ub = list(range(N8_XU, KT_X)) + list(range(KT_X + N8_HU, KT))
    sel_c8 = list(range(KT_X, KT))      # rh k-tiles (h rows of w_cand)
    sel_cb = list(range(KT_X))          # x k-tiles

    w_r8_np = np.stack([_slab(bg, t, kt_all, NP_FP8) for t in range(NT_C)])
    # merged fused-phase slabs: [u-gate k-tiles | cand k-tiles] per dtype
    w_uc8_np = np.stack([
        np.concatenate(
            [_slab(bg, NT_C + t, sel_u8, NP_FP8),
             _slab(bc, t, sel_c8, NP_FP8)], axis=1
        )
        for t in range(NT_C)
    ])
    w_ucb_np = np.stack([
        np.concatenate(
            [_slab(bg, NT_C + t, sel_ub, NP_BF16),
             _slab(bc, t, sel_cb, NP_BF16)], axis=1
        )
        for t in range(NT_C)
    ])

    b_g_np = np.ascontiguousarray(
        bias[: 2 * u].reshape(NT_G, P).T, dtype=np.float32
    )
    b_c_np = np.ascontiguousarray(
        bias[2 * u:].reshape(NT_C, P).T, dtype=np.float32
    )

    in_maps = []
    for i in range(N_CORES):
        sl = slice(i * B_LOC, (i + 1) * B_LOC)
        xT = x_t[sl].T  # [2048, 1024] fp32
        hT = h_tm1[sl].T
        in_maps.append(
            {
                "x8": np.ascontiguousarray(
                    xT.astype(NP_FP8).reshape(KT_X, P, B_LOC)
                ),
                "h8": np.ascontiguousarray(
                    hT.astype(NP_FP8).reshape(KT_H, P, B_LOC)
                ),
                "xb": np.ascontiguousarray(
                    xT.astype(NP_BF16).reshape(KT_X, P, B_LOC)
                ),
                "hb": np.ascontiguousarray(
                    hT.astype(NP_BF16).reshape(KT_H, P, B_LOC)
                ),
                "w_r8": w_r8_np,
                "w_uc8": w_uc8_np,
                "w_ucb": w_ucb_np,
                "b_g": b_g_np,
                "b_c": b_c_np,
            }
        )

    nc = _get_nc()
    res = run_bass_kernel_spmd(
        nc, in_maps, core_ids=list(range(N_CORES)), trace=TRACE
    )
    global LAST_RESULTS
    LAST_RESULTS = res

    h_t = np.empty((BATCH, UNITS), dtype=np.float32)
    for i in range(N_CORES):
        o = np.asarray(res.results[i]["out"], dtype=np.float32)
        h_t[i * B_LOC:(i + 1) * B_LOC] = o.reshape(UNITS, B_LOC).T
    return h_t


# revision 38
# speedup vs baseline: 1.0099x; 1.0099x over previous
"""GRU cell (AnotherGRUCell) on 8 TRN2 NeuronCores.

Strategy: pure data-parallel over batch (8192 rows -> 1024 rows/core),
weights replicated. No collectives.

All on-chip compute is in TRANSPOSED layout (units on the partition axis,
batch on the free axis), as in the bf16 baseline. New here: most of the
GEMM work runs in fp8-e4m3 with perf_mode=DoubleRow (2 k-tiles per PE
instruction), which roughly doubles PE matmul throughput. fp8
quantization noise is steered by a per-unit precision config chosen via
a host-side numpy simulation against the 2e-2 rel-err gate:

  - r gates (cols 0..15):     x@Wr + h@Wr fully fp8 (error is attenuated
                              through r*h -> cand -> (1-u) weighting)
  - u gates (cols 16..31):    first N8_XU/N8_HU k-tiles of x/h in fp8,
                              rest bf16 (u multiplies h directly in the
                              output, so u noise is expensive)
  - cand (r*h)@Wh3:           fully fp8 (attenuated like r)
  - cand x@Wi3:               bf16 (tanh pre-act noise is expensive)

All weights (both dtypes) are pre-scaled by S_W=32 on the host so fp8
sees a ~unit-std distribution, and every PSUM accumulation has one
uniform scale that is divided out for free inside the ScalarE
activation (out = sigmoid/tanh(psum * 1/S_W + bias)).

fp8 x/h/rh operands live in PAIR tiles [128, 2, 1024] so each DoubleRow
matmul gets its required 3D AP [128, 2, free] (pair-dim step % 16 == 0)
while startup DMAs keep per-pair dependency granularity.
"""

import numpy as np
import ml_dtypes

import concourse.bacc as bacc
import concourse.tile as tile
import concourse.mybir as mybir
from concourse.bass_utils import run_bass_kernel_spmd

N_CORES = 8
UNITS = 2048
IN_DIM = 2048
BATCH = 8192
B_LOC = BATCH // N_CORES  # 1024 batch rows per core

P = 128
KT_X = IN_DIM // P           # 16 k-tiles of x
KT_H = UNITS // P            # 16 k-tiles of h
KT = KT_X + KT_H             # 32 contraction k-tiles for [x; h]
NT_G = (2 * UNITS) // P      # 32 gate col-tiles (r: 0..15, u: 16..31)
NT_C = UNITS // P            # 16 candidate col-tiles
M_CHUNK = 512
MC = B_LOC // M_CHUNK        # 2 moving chunks per core

# Precision config: number of leading k-tiles (of 16) computed in fp8
# DoubleRow for the u-gate x/h operands. r gates and (r*h)@Wh3 are fully
# fp8; the candidate x@Wi3 is fully bf16. Must be even (DoubleRow pairs).
# Host-sim predicted rel err (matches HW to ~5 digits): 4 -> 1.571e-2,
# 8 -> 1.811e-2, 10 -> 1.920e-2 against the 2e-2 gate.
N8_XU = 10
N8_HU = 10
S_W = 32.0
S_INV = float(1.0 / S_W)

BF16 = mybir.dt.bfloat16
F32 = mybir.dt.float32
FP8 = mybir.dt.float8e4
NP_BF16 = ml_dtypes.bfloat16
NP_FP8 = ml_dtypes.float8_e4m3  # IEEE-style e4m3, max 240 == TRN FP8_EXP4
DR = mybir.MatmulPerfMode.DoubleRow

_CACHED_NC = None

# test.py sets TRACE=True to capture the NTFF profile (exec_time_ns +
# perfetto trace); the graded path leaves it off. LAST_RESULTS holds the
# BassKernelResults of the most recent run.
TRACE = False
LAST_RESULTS = None


def _build():
    nc = bacc.Bacc("TRN2", target_bir_lowering=False, debug=False)

    # fp8 transposed inputs, one [128, 1024] slice per k-tile
    x8d = nc.dram_tensor("x8", [KT_X, P, B_LOC], FP8, kind="ExternalInput")
    h8d = nc.dram_tensor("h8", [KT_H, P, B_LOC], FP8, kind="ExternalInput")
    # bf16 transposed inputs (u-gate bf16 part, cand x part, epilogues)
    xbd = nc.dram_tensor("xb", [KT_X, P, B_LOC], BF16, kind="ExternalInput")
    hbd = nc.dram_tensor("hb", [KT_H, P, B_LOC], BF16, kind="ExternalInput")
    # weights, pre-scaled by S_W, packed per col-tile as [128p, nkt, 128c]
    n8u = N8_XU + N8_HU
    nbu = KT - n8u
    w_r8 = nc.dram_tensor("w_r8", [NT_C, P, KT, P], FP8, kind="ExternalInput")
    # fused-phase slabs merged by dtype (one DMA + one PE first-use wait
    # per dtype per col-tile): fp8 = [u-gate fp8 k-tiles | rh k-tiles],
    # bf16 = [u-gate bf16 k-tiles | cand-x k-tiles]
    w_uc8 = nc.dram_tensor(
        "w_uc8", [NT_C, P, n8u + KT_H, P], FP8, kind="ExternalInput"
    )
    w_ucb = nc.dram_tensor(
        "w_ucb", [NT_C, P, nbu + KT_X, P], BF16, kind="ExternalInput"
    )
    # biases transposed: one [128, n_tiles] tensor per gate set -> 1 DMA each
    b_g = nc.dram_tensor("b_g", [P, NT_G], F32, kind="ExternalInput")
    b_c = nc.dram_tensor("b_c", [P, NT_C], F32, kind="ExternalInput")
    out = nc.dram_tensor("out", [NT_C, P, B_LOC], F32, kind="ExternalOutput")

    SIG = mybir.ActivationFunctionType.Sigmoid
    TANH = mybir.ActivationFunctionType.Tanh

    NPAIR_X = KT_X // 2
    NPAIR_H = KT_H // 2

    with tile.TileContext(nc) as tc:
        with (
            tc.tile_pool(name="resident", bufs=1) as res,
            tc.tile_pool(name="wslab", bufs=2) as wp,
            tc.tile_pool(name="psum", bufs=8, space="PSUM") as pp,
            tc.tile_pool(name="stage", bufs=2) as sp,
            tc.tile_pool(name="bias", bufs=1) as bp,
        ):
            # fp8 pair tiles: [128, 2, 1024]; pair q holds k-tiles 2q, 2q+1
            x8_pairs = [
                res.tile([P, 2, B_LOC], FP8, tag=f"x8{q}", name=f"x8{q}")
                for q in range(NPAIR_X)
            ]
            h8_pairs = [
                res.tile([P, 2, B_LOC], FP8, tag=f"h8{q}", name=f"h8{q}")
                for q in range(NPAIR_H)
            ]
            rh8_pairs = [
                res.tile([P, 2, B_LOC], FP8, tag=f"rh{q}", name=f"rh{q}")
                for q in range(NPAIR_H)
            ]
            # bf16 per-k-tile tiles
            xb_tiles = [
                res.tile([P, B_LOC], BF16, tag=f"xb{j}", name=f"xb{j}")
                for j in range(KT_X)
            ]
            hb_tiles = [
                res.tile([P, B_LOC], BF16, tag=f"hb{j}", name=f"hb{j}")
                for j in range(KT_H)
            ]
            # u gates are transient: phases U and C are interleaved per
            # col-tile, so u lives only from its sigmoid to the combine a
            # few us later (saves 30KB/partition of SBUF, spent on deeper
            # weight-slab prefetch and a wider startup interleave).

            # PE warm-up: the HAM clock gate holds the PE at 1.2 GHz until
            # it has been busy ~3.4us; fill the pre-first-matmul window
            # with dummy matmuls so the PE is un-throttled when real data
            # lands (same trick as the bf16 baseline).
            warm_src = sp.tile(
                [P, M_CHUNK], BF16, tag="warm", name="warm_src", bufs=1
            )
            nc.gpsimd.memset(warm_src[:], 0.0)
            warm_ps = pp.tile([P, M_CHUNK], F32, tag="psum", name="warm_ps")
            for w in range(8):
                nc.tensor.matmul(
                    warm_ps[:],
                    warm_src[:, :P],
                    warm_src[:],
                    start=(w == 0),
                    stop=(w == 7),
                )

            # Startup DMAs in exact consumption order of the first r-gate
            # col-tile pair, interleaved across both HWDGE rings.
            # Graduated chunk sizes (in k-tiles over the 32-long [x; h]
            # sequence); all chunk boundaries are even so DoubleRow pairs
            # never straddle a chunk.
            CHUNKS = [2, 6, 8, 8, 8]
            CB = [0, 2, 8, 16, 24, 32]  # chunk k-tile boundaries
            NT0 = 4  # r col-tiles in the startup block-interleave
            ws_first = [[None] * len(CHUNKS) for _ in range(NT0)]  # [t][chunk]
            src_dma = {}  # k-slot -> (engine, dst ap, src ap)
            for j in range(KT_X):
                eng = nc.sync if j % 2 == 0 else nc.scalar
                src_dma[j] = (eng, x8_pairs[j // 2][:, j % 2, :], x8d[j, :, :])
            for j in range(KT_H):
                eng = nc.scalar if j % 2 == 0 else nc.sync
                src_dma[KT_X + j] = (
                    eng, h8_pairs[j // 2][:, j % 2, :], h8d[j, :, :]
                )
            pre_ws = {}
            for c, cw in enumerate(CHUNKS):
                if c == 0:
                    # The very first matmul's operands go FIRST in each
                    # ring queue: x8 pair 0 then the first weight chunk.
                    for j in range(CB[0], CB[1]):
                        eng, dst, src = src_dma[j]
                        eng.dma_start(dst, src)
                if c == len(CHUNKS) - 1:
                    # Sneak the first steady-state r slabs in ahead of
                    # the last startup chunk: t=NT0's slab gates the PE
                    # right after the interleaved block and must not sit
                    # behind the bf16 input stream.
                    for t in (NT0, NT0 + 1):
                        ws = wp.tile([P, KT, P], FP8, tag="wr", name=f"wr{t}", bufs=3)
                        (nc.sync if t % 2 == 0 else nc.scalar).dma_start(
                            ws[:], w_r8[t, :, :, :]
                        )
                        pre_ws[t] = ws
                for t in range(NT0):
                    w = wp.tile(
                        [P, cw, P], FP8, tag=f"wr{t}_{c}", name=f"wr{t}_{c}",
                        bufs=1,
                    )
                    (nc.sync if t % 2 == 0 else nc.scalar).dma_start(
                        w[:], w_r8[t, :, CB[c]:CB[c + 1], :]
                    )
                    ws_first[t][c] = w
                if c > 0:
                    for j in range(CB[c], CB[c + 1]):
                        eng, dst, src = src_dma[j]
                        eng.dma_start(dst, src)

            # Biases + the early bf16 h tiles (needed by the first r
            # epilogues ~30us in) go on the SWDGE queue: the two HWDGE
            # rings deliver ~100GB/s each and are fully booked with the
            # startup x8/h8/weight traffic that gates the PE.
            bg_all = bp.tile([P, NT_G], F32, tag="bg", name="bg_all")
            nc.gpsimd.dma_start(bg_all[:], b_g[:, :])
            bc_all = bp.tile([P, NT_C], F32, tag="bc", name="bc_all")
            nc.gpsimd.dma_start(bc_all[:], b_c[:, :])
            for j in range(NT0 + 2):
                nc.gpsimd.dma_start(hb_tiles[j][:], hbd[j, :, :])

            all_pairs = x8_pairs + h8_pairs  # 16 fp8 pair tiles = 32 k-tiles

            def act_r(t, m, ps):
                """r epilogue: rh8[t] = sigmoid(ps/S_W + b) * h  (fp8)."""
                ms = slice(m * M_CHUNK, (m + 1) * M_CHUNK)
                rt = sp.tile([P, M_CHUNK], BF16, tag="rtmp", name=f"r{t}_{m}")
                nc.scalar.activation(
                    rt[:], ps[:], SIG, bias=bg_all[:, t:t + 1], scale=S_INV
                )
                nc.vector.tensor_mul(
                    rh8_pairs[t // 2][:, t % 2, ms], rt[:], hb_tiles[t][:, ms]
                )

            # ---- Phase R: r gates (cols 0..15), fully fp8 DoubleRow ------
            # The first NT0 col-tiles are block-interleaved over the
            # startup chunks (NT0*2 psum groups): the startup is input-
            # bandwidth-bound (~6MB before steady state), so the PE needs
            # ~NT0 tiles of matmul work per arriving chunk to stay busy.
            t0_groups = [(t, m) for t in range(NT0) for m in range(MC)]
            pss0 = [
                pp.tile([P, M_CHUNK], F32, tag="psum", name=f"psg0_{i}")
                for i in range(len(t0_groups))
            ]
            for c in range(len(CHUNKS)):
                q0, q1 = CB[c] // 2, CB[c + 1] // 2
                for i, (t, m) in enumerate(t0_groups):
                    ms = slice(m * M_CHUNK, (m + 1) * M_CHUNK)
                    for qq in range(q0, q1):
                        jj = qq - q0  # pair index within this chunk's slab
                        nc.tensor.matmul(
                            pss0[i][:],
                            ws_first[t][c][:, 2 * jj:2 * jj + 2, :],
                            all_pairs[qq][:, 0:2, ms],
                            start=(qq == 0),
                            stop=(qq == KT // 2 - 1),
                            perf_mode=DR,
                        )
            for i, (t, m) in enumerate(t0_groups):
                act_r(t, m, pss0[i])

            # Steady-state r cols: one fp8 slab [128, 32, 128] per col-tile,
            # m-interleaved so consecutive matmuls share the stationary
            # weight pair (one 256-col LDWEIGHTS per 2 matmuls).
            for t in range(NT0, NT_C):
                if t in pre_ws:
                    ws = pre_ws[t]
                else:
                    ws = wp.tile([P, KT, P], FP8, tag="wr", name=f"wr{t}", bufs=3)
                    (nc.sync if t % 2 == 0 else nc.scalar).dma_start(
                        ws[:], w_r8[t, :, :, :]
                    )
                # pace the bf16 inputs behind the slab they follow:
                # hb[t] lands ~1 col-tile before its epilogue needs it,
                # xb streams in over the back half of the r phase (it is
                # first read in the fused u/cand phase).
                if t < KT_H - 2:
                    (nc.scalar if t % 2 == 0 else nc.sync).dma_start(
                        hb_tiles[t + 2][:], hbd[t + 2, :, :]
                    )
                if t >= 8:
                    j0 = 2 * (t - 8)
                    (nc.scalar if t % 2 == 0 else nc.sync).dma_start(
                        xb_tiles[j0][:], xbd[j0, :, :]
                    )
                    (nc.sync if t % 2 == 0 else nc.scalar).dma_start(
                        xb_tiles[j0 + 1][:], xbd[j0 + 1, :, :]
                    )
                psl = [
                    pp.tile([P, M_CHUNK], F32, tag="psum", name=f"psr{t}_{m}")
                    for m in range(MC)
                ]
                for q in range(KT // 2):
                    for m in range(MC):
                        ms = slice(m * M_CHUNK, (m + 1) * M_CHUNK)
                        nc.tensor.matmul(
                            psl[m][:],
                            ws[:, 2 * q:2 * q + 2, :],
                            all_pairs[q][:, 0:2, ms],
                            start=(q == 0),
                            stop=(q == KT // 2 - 1),
                            perf_mode=DR,
                        )
                for m in range(MC):
                    act_r(t, m, psl[m])

            # ---- Fused phase U+C: per col-tile t, compute the u gate
            # (cols 16+t, mixed fp8/bf16) and immediately the candidate +
            # output combine for the same t. u_t lives only a few us in a
            # rotating stage tile instead of 32KB of resident SBUF.
            # psum_c = (r*h)@Wh3 (fp8 DR) + x@Wi3 (bf16);
            # h_t = u * (h - cand) + cand
            def uc_slabs(t):
                w8t = wp.tile(
                    [P, n8u + KT_H, P], FP8, tag="wuc8", name=f"wuc8_{t}",
                    bufs=3,
                )
                (nc.sync if t % 2 == 0 else nc.scalar).dma_start(
                    w8t[:], w_uc8[t, :, :, :]
                )
                wbt = wp.tile(
                    [P, nbu + KT_X, P], BF16, tag="wucb", name=f"wucb_{t}",
                    bufs=3,
                )
                (nc.scalar if t % 2 == 0 else nc.sync).dma_start(
                    wbt[:], w_ucb[t, :, :, :]
                )
                return w8t, wbt

            # The fused tile emits all DoubleRow work (u then cand, both
            # from the one merged fp8 slab), then all bf16 work (one
            # merged bf16 slab): the PE pays ~420ns at each DR<->Normal
            # weight-path transition, so 2 transitions per tile, not 4.
            def u_accum_dr(w8, psl, stop):
                step = 0
                last = (N8_XU + N8_HU) // 2 - 1
                for q in range(N8_XU // 2):
                    for m in range(MC):
                        ms = slice(m * M_CHUNK, (m + 1) * M_CHUNK)
                        nc.tensor.matmul(
                            psl[m][:],
                            w8[:, 2 * q:2 * q + 2, :],
                            x8_pairs[q][:, 0:2, ms],
                            start=(step == 0),
                            stop=(stop and step == last),
                            perf_mode=DR,
                        )
                    step += 1
                for q in range(N8_HU // 2):
                    off = N8_XU + 2 * q
                    for m in range(MC):
                        ms = slice(m * M_CHUNK, (m + 1) * M_CHUNK)
                        nc.tensor.matmul(
                            psl[m][:],
                            w8[:, off:off + 2, :],
                            h8_pairs[q][:, 0:2, ms],
                            start=(step == 0),
                            stop=(stop and step == last),
                            perf_mode=DR,
                        )
                    step += 1

            def u_accum_bf(wb, psl):
                srcs = (
                    [xb_tiles[j] for j in range(N8_XU, KT_X)]
                    + [hb_tiles[j] for j in range(N8_HU, KT_H)]
                )
                for i, src in enumerate(srcs):
                    for m in range(MC):
                        ms = slice(m * M_CHUNK, (m + 1) * M_CHUNK)
                        nc.tensor.matmul(
                            psl[m][:],
                            wb[:, i, :],
                            src[:, ms],
                            start=False,
                            stop=(i == len(srcs) - 1),
                        )

            def cand_accum_dr(w8, psl):
                for q in range(KT_H // 2):
                    off = n8u + 2 * q
                    for m in range(MC):
                        ms = slice(m * M_CHUNK, (m + 1) * M_CHUNK)
                        nc.tensor.matmul(
                            psl[m][:],
                            w8[:, off:off + 2, :],
                            rh8_pairs[q][:, 0:2, ms],
                            start=(q == 0),
                            stop=False,
                            perf_mode=DR,
                        )

            def cand_accum_bf(wb, psl):
                for j in range(KT_X):
                    for m in range(MC):
                        ms = slice(m * M_CHUNK, (m + 1) * M_CHUNK)
                        nc.tensor.matmul(
                            psl[m][:],
                            wb[:, nbu + j, :],
                            xb_tiles[j][:, ms],
                            start=False,
                            stop=(j == KT_X - 1),
                        )

            def cand_epilogue(t, m, ut, ps):
                ms = slice(m * M_CHUNK, (m + 1) * M_CHUNK)
                cand = sp.tile([P, M_CHUNK], F32, tag="cand", name=f"c{t}_{m}")
                nc.scalar.activation(
                    cand[:], ps[:], TANH, bias=bc_all[:, t:t + 1], scale=S_INV
                )
                d = sp.tile([P, M_CHUNK], F32, tag="d", name=f"d{t}_{m}")
                nc.vector.tensor_sub(d[:], hb_tiles[t][:, ms], cand[:])
                d2 = sp.tile([P, M_CHUNK], F32, tag="d2", name=f"d2{t}_{m}")
                nc.vector.tensor_mul(d2[:], ut[:, ms], d[:])
                ht = sp.tile([P, M_CHUNK], F32, tag="ht", name=f"ht{t}_{m}")
                nc.vector.tensor_add(ht[:], d2[:], cand[:])
                # Outs split across both rings; tile t+1's slab DMAs are
                # issued BEFORE these in program order, so outputs never
                # delay the weight stream (run-2's 13us tail) and don't
                # drain on the slow SWDGE queue (run-3's 17us tail).
                (nc.sync if m == 0 else nc.scalar).dma_start(
                    out[t, :, ms], ht[:]
                )

            def cand_epilogue_narrow(t, m, half, ut, ps):
                """256-wide epilogue slice: on the last tile the ACT->DVE
                ->DMA chain is the post-final-matmul drain, so pipeline it
                at half-chunk granularity to shorten the critical path."""
                HW = M_CHUNK // 2
                ms = slice(m * M_CHUNK + half * HW, m * M_CHUNK + (half + 1) * HW)
                ps_sl = ps[:, half * HW:(half + 1) * HW]
                cand = sp.tile([P, HW], F32, tag="cand", name=f"cn{m}_{half}")
                nc.scalar.activation(
                    cand[:], ps_sl, TANH, bias=bc_all[:, t:t + 1], scale=S_INV
                )
                d = sp.tile([P, HW], F32, tag="d", name=f"dn{m}_{half}")
                nc.vector.tensor_sub(d[:], hb_tiles[t][:, ms], cand[:])
                d2 = sp.tile([P, HW], F32, tag="d2", name=f"d2n{m}_{half}")
                nc.vector.tensor_mul(d2[:], ut[:, ms], d[:])
                ht = sp.tile([P, HW], F32, tag="ht", name=f"htn{m}_{half}")
                nc.vector.tensor_add(ht[:], d2[:], cand[:])
                (nc.sync if half == 0 else nc.scalar).dma_start(
                    out[t, :, ms], ht[:]
                )

            slabs = {0: uc_slabs(0)}
            for t in range(NT_C):
                if t + 1 < NT_C:
                    slabs[t + 1] = uc_slabs(t + 1)
                w8t, wbt = slabs.pop(t)
                ut = sp.tile([P, B_LOC], BF16, tag="ut", name=f"ut{t}")
                psu = [
                    pp.tile([P, M_CHUNK], F32, tag="psum", name=f"psu{t}_{m}")
                    for m in range(MC)
                ]
                psc = [
                    pp.tile([P, M_CHUNK], F32, tag="psum", name=f"psc{t}_{m}")
                    for m in range(MC)
                ]
                u_accum_dr(w8t, psu, stop=False)
                cand_accum_dr(w8t, psc)
                u_accum_bf(wbt, psu)
                for m in range(MC):
                    ms = slice(m * M_CHUNK, (m + 1) * M_CHUNK)
                    nc.scalar.activation(
                        ut[:, ms], psu[m][:], SIG,
                        bias=bg_all[:, NT_C + t:NT_C + t + 1], scale=S_INV,
                    )
                cand_accum_bf(wbt, psc)
                if t < NT_C - 1:
                    for m in range(MC):
                        cand_epilogue(t, m, ut, psc[m])
                else:
                    for m in range(MC):
                        for half in range(2):
                            cand_epilogue_narrow(t, m, half, ut, psc[m])

    nc.compile()
    return nc


def _get_nc():
    global _CACHED_NC
    if _CACHED_NC is None:
        _CACHED_NC = _build()
    return _CACHED_NC


def _ct_blocks(w):
    """[K, N] -> [N/128 col-tiles, K/128 k-tiles, 128p, 128c] blocks."""
    K, N = w.shape
    return np.ascontiguousarray(
        w.reshape(K // P, P, N // P, P).transpose(2, 0, 1, 3)
    )


def _slab(blocks, ct, sel, np_dtype):
    """Pack k-tiles `sel` of col-tile ct into [128p, len(sel), 128c]."""
    a = blocks[ct][sel]  # [nkt, 128p, 128c]
    return np.ascontiguousarray(a.transpose(1, 0, 2)).astype(np_dtype)


def kernel(x_t, h_tm1, input_weight, hidden_state_weight, bias):
    x_t = np.asarray(x_t, dtype=np.float32)
    h_tm1 = np.asarray(h_tm1, dtype=np.float32)
    input_weight = np.asarray(input_weight, dtype=np.float32)
    hidden_state_weight = np.asarray(hidden_state_weight, dtype=np.float32)
    bias = np.asarray(bias, dtype=np.float32)

    u = UNITS
    # Gate weights: [x; h] @ [Wi[:, :2u]; Wh[:, :2u]], pre-scaled by S_W
    w_gate = np.concatenate(
        [input_weight[:, : 2 * u], hidden_state_weight[:, : 2 * u]], axis=0
    ) * np.float32(S_W)  # [4096, 4096]
    w_cand = np.concatenate(
        [input_weight[:, 2 * u:], hidden_state_weight[:, 2 * u:]], axis=0
    ) * np.float32(S_W)  # [4096, 2048]

    bg = _ct_blocks(w_gate)   # [32 ct, 32 kt, 128, 128]
    bc = _ct_blocks(w_cand)   # [16 ct, 32 kt, 128, 128]

    kt_all = list(range(KT))
    sel_u8 = list(range(N8_XU)) + list(range(KT_X, KT_X + N8_HU))
    sel_ub = list(range(N8_XU, KT_X)) + list(range(KT_X + N8_HU, KT))
    sel_c8 = list(range(KT_X, KT))      # rh k-tiles (h rows of w_cand)
    sel_cb = list(range(KT_X))          # x k-tiles

    w_r8_np = np.stack([_slab(bg, t, kt_all, NP_FP8) for t in range(NT_C)])
    # merged fused-phase slabs: [u-gate k-tiles | cand k-tiles] per dtype
    w_uc8_np = np.stack([
        np.concatenate(
            [_slab(bg, NT_C + t, sel_u8, NP_FP8),
             _slab(bc, t, sel_c8, NP_FP8)], axis=1
        )
        for t in range(NT_C)
    ])
    w_ucb_np = np.stack([
        np.concatenate(
            [_slab(bg, NT_C + t, sel_ub, NP_BF16),
             _slab(bc, t, sel_cb, NP_BF16)], axis=1
        )
        for t in range(NT_C)
    ])

    b_g_np = np.ascontiguousarray(
        bias[: 2 * u].reshape(NT_G, P).T, dtype=np.float32
    )
    b_c_np = np.ascontiguousarray(
        bias[2 * u:].reshape(NT_C, P).T, dtype=np.float32
    )

    in_maps = []
    for i in range(N_CORES):
        sl = slice(i * B_LOC, (i + 1) * B_LOC)
        xT = x_t[sl].T  # [2048, 1024] fp32
        hT = h_tm1[sl].T
        in_maps.append(
            {
                "x8": np.ascontiguousarray(
                    xT.astype(NP_FP8).reshape(KT_X, P, B_LOC)
                ),
                "h8": np.ascontiguousarray(
                    hT.astype(NP_FP8).reshape(KT_H, P, B_LOC)
                ),
                "xb": np.ascontiguousarray(
                    xT.astype(NP_BF16).reshape(KT_X, P, B_LOC)
                ),
                "hb": np.ascontiguousarray(
                    hT.astype(NP_BF16).reshape(KT_H, P, B_LOC)
                ),
                "w_r8": w_r8_np,
                "w_uc8": w_uc8_np,
                "w_ucb": w_ucb_np,
                "b_g": b_g_np,
                "b_c": b_c_np,
            }
        )

    nc = _get_nc()
    res = run_bass_kernel_spmd(
        nc, in_maps, core_ids=list(range(N_CORES)), trace=TRACE
    )
    global LAST_RESULTS
    LAST_RESULTS = res

    h_t = np.empty((BATCH, UNITS), dtype=np.float32)
    for i in range(N_CORES):
        o = np.asarray(res.results[i]["out"], dtype=np.float32)
        h_t[i * B_LOC:(i + 1) * B_LOC] = o.reshape(UNITS, B_LOC).T
    return h_t
